# revision 16
# baseline (speedup 1.0000x reference)
"""Trainium2 Bass kernel for nn_AttentionHead (additive/Bahdanau attention).

reference:
    kt = einsum('bkh,oh->bko', x_key, w1)          # (B, NK, H)
    qt = einsum('bqh,oh->bqo', x_query, w2)        # (B, NQ, H)
    prod[b,q,k] = sum_h v[h] * tanh(kt[b,k,h] + qt[b,q,h])
    out = log_softmax(prod, axis=-1)               # (B, NQ, NK)

Shapes: B=4, NQ=256, NK=512, H=256.  8 NeuronCores, data-parallel over
(B x NQ/2): core c handles b = c//2 and a 128-row slice of NQ.

Per-core dataflow:
  - host marshals packed fp32 inputs: transposed xk, xq, w1, w2 plus the
    "ediag" stationaries (for each (h_tile, j in 0..31) a (128,32) matrix,
    zero except column j = v[h_tile*128 : +128]).
  - PE: ktT[o_t] (128, 512) = w1T.T @ xkT       (o on partitions, k free)
        qtT[o_t] (128, 128) = w2T.T @ xqT       (o on partitions, q free)
        ktT cast to bf16 on the PSUM->SBUF copy.
  - DVE: S[h_t][:, q*512:+512] = ktT[h_t] + qtT[h_t][:, q]  (bf16 in/out,
    fp32 per-partition scalar -> high DVE perf mode)
  - ACT: tanh in place on S in large (128, 8192) instructions (the kernel
    bottleneck: 134M tanh elements live on ScalarE only)
  - PE:  prod[q, :] += ediag[h_t, q%32].T @ S[h_t][:, q]  via col-tiled
    matmuls (tile_position=(0, 32j)) accumulating into one PSUM bank
    (q on partitions, k free)
  - log_softmax along free axis, regionized over 32-q row blocks so most
    of it overlaps the main loop: out = prod - ln(sum_k exp(prod));
    |prod| <= sum|v| ~ 8 so skipping max-subtraction is safe in fp32.

walrus only supports ONE sync wait per instruction: split_multi_waits()
post-processes the scheduled IR, moving extra waits onto same-engine
NoOps inserted immediately before the offending instruction.
"""

import sys

sys.path.insert(0, "/opt/trn_rl_repo")

import numpy as np
import ml_dtypes

import concourse.bass as bass
import concourse.mybir as mybir
from concourse import tile
from concourse.bass_utils import run_bass_kernel_spmd

F32 = mybir.dt.float32
BF16 = mybir.dt.bfloat16
AF = mybir.ActivationFunctionType
ALU = mybir.AluOpType

B, NQ, NK, H = 4, 256, 512, 256
NCORES = 8
QPC = (B * NQ) // NCORES  # 128 q rows per core
GROUP = 16                # q's per pipeline group
NGROUPS = QPC // GROUP    # 8

PKK_F = 1536              # xkT (2x512) | w1T (2x256)
PKQ_F = 768               # xqT (2x128) | w2T (2x256)
ED_F = 2 * 32 * 32        # (128, 2048) bf16 v-diag stationaries


def build_program(split=True):
    nc = bass.Bass()

    pkk_d = nc.dram_tensor("packed_k", (128, PKK_F), BF16, kind="ExternalInput")
    pkq_d = nc.dram_tensor("packed_q", (128, PKQ_F), BF16, kind="ExternalInput")
    ed_d = nc.dram_tensor("ediag", (128, ED_F), BF16, kind="ExternalInput")
    out_d = nc.dram_tensor("out", (QPC, NK), F32, kind="ExternalOutput")

    from concourse.tile_rust import add_dep_helper

    with tile.TileContext(nc) as tc:
        with (
            tc.tile_pool(name="const", bufs=1) as cpool,
            tc.tile_pool(name="sadd", bufs=2) as spool,
            tc.tile_pool(name="ppre", bufs=2, space="PSUM") as ppool,
            tc.tile_pool(name="prod", bufs=1, space="PSUM") as prodpool,
        ):
            packed_k = cpool.tile([128, PKK_F], BF16, tag="packed_k")
            packed_q = cpool.tile([128, PKQ_F], BF16, tag="packed_q")
            ed_bf = cpool.tile([128, ED_F], BF16, tag="ed_bf")
            # chain the input DMAs so each gets full HBM bandwidth in
            # criticality order (kt inputs -> qt inputs -> ediag)
            d1 = nc.sync.dma_start(packed_q[:], pkq_d[:])
            d2 = nc.sync.dma_start(packed_k[:], pkk_d[:])
            d3 = nc.sync.dma_start(ed_bf[:], ed_d[:])
            add_dep_helper(d2.ins, d1.ins, True, "serialize input DMAs")
            add_dep_helper(d3.ins, d2.ins, True, "serialize input DMAs")

            def xkT(i):
                return packed_k[:, i * NK:(i + 1) * NK]

            def w1T(i, o):
                return packed_k[:, 1024 + i * 256 + o * 128:1024 + i * 256 + (o + 1) * 128]

            def xqT(i):
                return packed_q[:, i * QPC:(i + 1) * QPC]

            def w2T(i, o):
                return packed_q[:, 256 + i * 256 + o * 128:256 + i * 256 + (o + 1) * 128]

            # ---- ktT / qtT (PSUM->SBUF copies on the idle ACT engine) -------
            ktT_sb = [cpool.tile([128, NK], BF16, tag=f"ktT{o}", name=f"ktT{o}")
                      for o in range(2)]
            qtT_sb = [cpool.tile([128, QPC], F32, tag=f"qtT{o}", name=f"qtT{o}")
                      for o in range(2)]
            # o_t-major so the o=0 results (all the first tanh chunk needs)
            # land first; o=1 matmuls/copies overlap the first S0 adds.
            for o_t in range(2):
                pq = ppool.tile([128, QPC], F32, tag="pq", name="pq")
                for h_t in range(2):
                    nc.tensor.matmul(
                        pq[:], w2T(h_t, o_t), xqT(h_t),
                        start=(h_t == 0), stop=(h_t == 1),
                    )
                pk = ppool.tile([128, NK], F32, tag="pk", name="pk")
                for h_t in range(2):
                    nc.tensor.matmul(
                        pk[:], w1T(h_t, o_t), xkT(h_t),
                        start=(h_t == 0), stop=(h_t == 1),
                    )
                if o_t == 0:
                    nc.vector.tensor_copy(qtT_sb[0][:], pq[:])
                    nc.scalar.copy(ktT_sb[0][:], pk[:])
                else:
                    nc.scalar.copy(qtT_sb[1][:], pq[:])
                    nc.scalar.copy(ktT_sb[1][:], pk[:])

            # ---- main loop ---------------------------------------------------
            # Two PSUM banks for prod: q 0..63 in bank A partitions 0..63,
            # q 64..127 in bank B partitions 64..127 -> the bank-A softmax
            # overlaps the bank-B matmuls (PSUM deps are bank-granular).
            prodA = prodpool.tile([128, NK], F32, tag="prodA", name="prodA")
            prodB = prodpool.tile([128, NK], F32, tag="prodB", name="prodB")
            sumexp = cpool.tile([128, 1], F32, tag="sumexp")
            lse = cpool.tile([128, 1], F32, tag="lse")
            neg_lse = cpool.tile([128, 1], F32, tag="neg_lse")
            expt = cpool.tile([128, NK], F32, tag="expt")
            out_sb = cpool.tile([128, NK], F32, tag="out_sb")

            for g in range(NGROUPS):
                prod = prodA if g < 4 else prodB
                S = [spool.tile([128, GROUP * NK], BF16, tag=f"S{i}", name=f"S{i}")
                     for i in range(2)]
                for h_t in range(2):
                    for ql in range(GROUP):
                        q = g * GROUP + ql
                        nc.vector.tensor_scalar(
                            S[h_t][:, ql * NK:(ql + 1) * NK],
                            ktT_sb[h_t][:],
                            qtT_sb[h_t][:, q:q + 1],
                            None,
                            op0=ALU.add,
                        )
                if g == 0:
                    # fill the pipeline sooner: first tanh needs only 4 adds
                    for cs, ce in ((0, 4), (4, 10), (10, GROUP)):
                        nc.scalar.activation(
                            S[0][:, cs * NK:ce * NK],
                            S[0][:, cs * NK:ce * NK], AF.Tanh,
                        )
                    nc.scalar.activation(S[1][:], S[1][:], AF.Tanh)
                elif g == NGROUPS - 1:
                    # drain the pipeline sooner: matmuls trail each half
                    nc.scalar.activation(S[0][:], S[0][:], AF.Tanh)
                    for cs, ce in ((0, 8), (8, GROUP)):
                        nc.scalar.activation(
                            S[1][:, cs * NK:ce * NK],
                            S[1][:, cs * NK:ce * NK], AF.Tanh,
                        )
                else:
                    for h_t in range(2):
                        nc.scalar.activation(S[h_t][:], S[h_t][:], AF.Tanh)
                for h_t in range(2):
                    for ql in range(GROUP):
                        q = g * GROUP + ql
                        j = (q // 32) * 32
                        jj = q % 32
                        nc.tensor.matmul(
                            prod[j:j + 32, :],
                            ed_bf[:, h_t * 1024 + jj * 32: h_t * 1024 + jj * 32 + 32],
                            S[h_t][:, ql * NK:(ql + 1) * NK],
                            start=(jj == 0 and h_t == 0),
                            stop=(jj == 31 and h_t == 1),
                            tile_position=(0, j),
                        )
                if g == 4:
                    # bank A finished at end of group 3; its exp overlaps
                    # bank B's matmuls (emitted here so it slots between
                    # group 4 and group 5 tanh work on the ACT queue)
                    nc.scalar.activation(
                        expt[0:64, :], prodA[0:64, :], AF.Exp,
                        accum_out=sumexp[0:64, :],
                    )

            # ---- log_softmax tail (bank B + shared ln/identity) -------------
            nc.scalar.activation(
                expt[64:128, :], prodB[64:128, :], AF.Exp,
                accum_out=sumexp[64:128, :],
            )
            nc.scalar.activation(lse[:], sumexp[:], AF.Ln)
            nc.vector.tensor_scalar_mul(neg_lse[:], lse[:], -1.0)
            nc.scalar.activation(
                out_sb[0:64, :], prodA[0:64, :], AF.Identity,
                bias=neg_lse[0:64, 0:1],
            )
            nc.scalar.activation(
                out_sb[64:128, :], prodB[64:128, :], AF.Identity,
                bias=neg_lse[64:128, 0:1],
            )
            nc.sync.dma_start(out_d[:], out_sb[:])

    if split:
        split_multi_waits(nc)
    return nc


def split_multi_waits(nc):
    """walrus codegen accepts at most one sync wait per instruction; move
    extra waits onto same-engine NoOps inserted immediately before."""
    n = 0
    for fn in nc.m.functions:
        for blk in fn.blocks:
            new_insts = []
            for inst in blk.instructions:
                si = inst.sync_info
                if si is not None and len(si.on_wait) > 1:
                    waits = list(si.on_wait)
                    for w in waits[:-1]:
                        nop = mybir.InstNoOp(name=f"WSPLIT-{n}", ins=[], outs=[])
                        n += 1
                        nop.engine = inst.engine
                        nop.sync_info = mybir.SyncInfo(on_wait=[w], on_update=[])
                        new_insts.append(nop)
                    inst.sync_info = mybir.SyncInfo(
                        on_wait=[waits[-1]], on_update=list(si.on_update)
                    )
                new_insts.append(inst)
            if n:
                blk.instructions = new_insts
    return n


def audit_waits(nc, max_waits=1):
    bad = []
    for fn in nc.m.functions:
        for blk in fn.blocks:
            for inst in blk.instructions:
                si = inst.sync_info
                if si is not None and len(si.on_wait) > max_waits:
                    bad.append((inst.name, type(inst).__name__,
                                [w.ant_name for w in si.on_wait]))
    return bad


def make_in_maps(x_query, x_key, w1, w2, v):
    x_query = np.asarray(x_query, dtype=np.float32)
    x_key = np.asarray(x_key, dtype=np.float32)
    w1 = np.asarray(w1, dtype=np.float32)
    w2 = np.asarray(w2, dtype=np.float32)
    v = np.asarray(v, dtype=np.float32).reshape(H)

    w1T = np.ascontiguousarray(w1.T)  # (h_in, o)
    w2T = np.ascontiguousarray(w2.T)

    # ediag[p, h_t*1024 + j*32 + c] = v[h_t*128 + p] if c == j else 0
    ed = np.zeros((128, 2, 32, 32), dtype=np.float32)
    for h_t in range(2):
        for j in range(32):
            ed[:, h_t, j, j] = v[h_t * 128:(h_t + 1) * 128]
    ed = np.ascontiguousarray(ed.reshape(128, ED_F).astype(ml_dtypes.bfloat16))

    in_maps = []
    for c in range(NCORES):
        b = c // 2
        q0 = (c % 2) * QPC
        xqT = np.ascontiguousarray(x_query[b, q0:q0 + QPC, :].T)  # (H, 128)
        xkT = np.ascontiguousarray(x_key[b].T)                    # (H, 512)
        packed_k = np.concatenate(
            [xkT[:128], xkT[128:], w1T[:128], w1T[128:]], axis=1)
        packed_q = np.concatenate(
            [xqT[:128], xqT[128:], w2T[:128], w2T[128:]], axis=1)
        assert packed_k.shape == (128, PKK_F)
        assert packed_q.shape == (128, PKQ_F)
        in_maps.append({
            "packed_k": np.ascontiguousarray(packed_k.astype(ml_dtypes.bfloat16)),
            "packed_q": np.ascontiguousarray(packed_q.astype(ml_dtypes.bfloat16)),
            "ediag": ed,
        })
    return in_maps


_prog_cache = {}


def kernel(x_query, x_key, w1, w2, v):
    if "nc" not in _prog_cache:
        _prog_cache["nc"] = build_program()
    nc = _prog_cache["nc"]
    in_maps = make_in_maps(x_query, x_key, w1, w2, v)
    # A previously-profiled session can leave the device wedged; the failed
    # attempt resets it, so retry a couple of times.
    last_err = None
    for _ in range(3):
        try:
            res = run_bass_kernel_spmd(nc, in_maps, list(range(NCORES)))
            break
        except Exception as e:  # noqa: BLE001 - NRT_EXEC_UNIT_UNRECOVERABLE etc
            last_err = e
    else:
        raise last_err
    out = np.empty((B, NQ, NK), dtype=np.float32)
    for c in range(NCORES):
        b = c // 2
        q0 = (c % 2) * QPC
        out[b, q0:q0 + QPC, :] = res.results[c]["out"]
    return out


if __name__ == "__main__":
    nc = build_program()
    bad = audit_waits(nc)
    if bad:
        print(f"{len(bad)} instructions exceed the 1-wait budget:")
        for name, ty, waits in bad[:20]:
            print(" ", name, ty, waits)
    else:
        print("wait audit OK: all instructions <= 1 sync wait")


# revision 17
# speedup vs baseline: 1.1906x; 1.1906x over previous
"""Trainium2 Bass kernel for nn_AttentionHead (additive/Bahdanau attention).

reference:
    kt = einsum('bkh,oh->bko', x_key, w1)          # (B, NK, H)
    qt = einsum('bqh,oh->bqo', x_query, w2)        # (B, NQ, H)
    prod[b,q,k] = sum_h v[h] * tanh(kt[b,k,h] + qt[b,q,h])
    out = log_softmax(prod, axis=-1)               # (B, NQ, NK)

Shapes: B=4, NQ=256, NK=512, H=256.  8 NeuronCores, data-parallel over
(B x NQ/2): core c handles b = c//2 and a 128-row slice of NQ.

Per-core dataflow:
  - host marshals packed fp32 inputs: transposed xk, xq, w1, w2 plus the
    "ediag" stationaries (for each (h_tile, j in 0..31) a (128,32) matrix,
    zero except column j = v[h_tile*128 : +128]).
  - PE: ktT[o_t] (128, 512) = w1T.T @ xkT       (o on partitions, k free)
        qtT[o_t] (128, 128) = w2T.T @ xqT       (o on partitions, q free)
        ktT cast to bf16 on the PSUM->SBUF copy.
  - DVE: S[h_t][:, q*512:+512] = ktT[h_t] + qtT[h_t][:, q]  (bf16 in/out,
    fp32 per-partition scalar -> high DVE perf mode)
  - ACT: tanh in place on S in large (128, 8192) instructions (the kernel
    bottleneck: 134M tanh elements live on ScalarE only)
  - PE:  prod[q, :] += ediag[h_t, q%32].T @ S[h_t][:, q]  via col-tiled
    matmuls (tile_position=(0, 32j)) accumulating into one PSUM bank
    (q on partitions, k free)
  - log_softmax along free axis, regionized over 32-q row blocks so most
    of it overlaps the main loop: out = prod - ln(sum_k exp(prod));
    |prod| <= sum|v| ~ 8 so skipping max-subtraction is safe in fp32.

walrus only supports ONE sync wait per instruction: split_multi_waits()
post-processes the scheduled IR, moving extra waits onto same-engine
NoOps inserted immediately before the offending instruction.
"""

import sys

sys.path.insert(0, "/opt/trn_rl_repo")

import numpy as np
import ml_dtypes

import concourse.bass as bass
import concourse.mybir as mybir
from concourse import tile
from concourse.bass_utils import run_bass_kernel_spmd

F32 = mybir.dt.float32
BF16 = mybir.dt.bfloat16
AF = mybir.ActivationFunctionType
ALU = mybir.AluOpType

B, NQ, NK, H = 4, 256, 512, 256
NCORES = 8
QPC = (B * NQ) // NCORES  # 128 q rows per core
GROUP = 16                # q's per pipeline group
NGROUPS = QPC // GROUP    # 8

PKK_F = 1536              # xkT (2x512) | w1T (2x256)
PKQ_F = 768               # xqT (2x128) | w2T (2x256)
ED_F = 2 * 32 * 32        # (128, 2048) bf16 v-diag stationaries


def build_program(split=True):
    nc = bass.Bass()

    pkk_d = nc.dram_tensor("packed_k", (128, PKK_F), BF16, kind="ExternalInput")
    pkq_d = nc.dram_tensor("packed_q", (128, PKQ_F), BF16, kind="ExternalInput")
    ed_d = nc.dram_tensor("ediag", (128, ED_F), BF16, kind="ExternalInput")
    out_d = nc.dram_tensor("out", (QPC, NK), F32, kind="ExternalOutput")

    from concourse.tile_rust import add_dep_helper

    with tile.TileContext(nc) as tc:
        with (
            tc.tile_pool(name="const", bufs=1) as cpool,
            tc.tile_pool(name="sadd", bufs=2) as spool,
            tc.tile_pool(name="ppre", bufs=2, space="PSUM") as ppool,
            tc.tile_pool(name="prod", bufs=1, space="PSUM") as prodpool,
        ):
            packed_k = cpool.tile([128, PKK_F], BF16, tag="packed_k")
            packed_q = cpool.tile([128, PKQ_F], BF16, tag="packed_q")
            ed_bf = cpool.tile([128, ED_F], BF16, tag="ed_bf")
            # chain the input DMAs so each gets full HBM bandwidth in
            # criticality order (kt inputs -> qt inputs -> ediag)
            d1 = nc.sync.dma_start(packed_q[:], pkq_d[:])
            d2 = nc.sync.dma_start(packed_k[:], pkk_d[:])
            d3 = nc.sync.dma_start(ed_bf[:], ed_d[:])
            add_dep_helper(d2.ins, d1.ins, True, "serialize input DMAs")
            add_dep_helper(d3.ins, d2.ins, True, "serialize input DMAs")

            def xkT(i):
                return packed_k[:, i * NK:(i + 1) * NK]

            def w1T(i, o):
                return packed_k[:, 1024 + i * 256 + o * 128:1024 + i * 256 + (o + 1) * 128]

            def xqT(i):
                return packed_q[:, i * QPC:(i + 1) * QPC]

            def w2T(i, o):
                return packed_q[:, 256 + i * 256 + o * 128:256 + i * 256 + (o + 1) * 128]

            # ---- ktT / qtT (PSUM->SBUF copies on the idle ACT engine) -------
            ktT_sb = [cpool.tile([128, NK], BF16, tag=f"ktT{o}", name=f"ktT{o}")
                      for o in range(2)]
            qtT_sb = [cpool.tile([128, QPC], F32, tag=f"qtT{o}", name=f"qtT{o}")
                      for o in range(2)]
            for o_t in range(2):
                pq = ppool.tile([128, QPC], F32, tag="pq", name="pq")
                for h_t in range(2):
                    nc.tensor.matmul(
                        pq[:], w2T(h_t, o_t), xqT(h_t),
                        start=(h_t == 0), stop=(h_t == 1),
                    )
                nc.vector.tensor_copy(qtT_sb[o_t][:], pq[:])
            for o_t in range(2):
                pk = ppool.tile([128, NK], F32, tag="pk", name="pk")
                for h_t in range(2):
                    nc.tensor.matmul(
                        pk[:], w1T(h_t, o_t), xkT(h_t),
                        start=(h_t == 0), stop=(h_t == 1),
                    )
                nc.vector.tensor_copy(ktT_sb[o_t][:], pk[:])

            # ---- main loop ---------------------------------------------------
            # Two PSUM banks for prod: q 0..63 in bank A partitions 0..63,
            # q 64..127 in bank B partitions 64..127 -> the bank-A softmax
            # overlaps the bank-B matmuls (PSUM deps are bank-granular).
            prodA = prodpool.tile([128, NK], F32, tag="prodA", name="prodA")
            prodB = prodpool.tile([128, NK], F32, tag="prodB", name="prodB")
            sumexp = cpool.tile([128, 1], F32, tag="sumexp")
            lse = cpool.tile([128, 1], F32, tag="lse")
            neg_lse = cpool.tile([128, 1], F32, tag="neg_lse")
            expt = cpool.tile([128, NK], F32, tag="expt")
            out_sb = cpool.tile([128, NK], F32, tag="out_sb")

            for g in range(NGROUPS):
                prod = prodA if g < 4 else prodB
                S = [spool.tile([128, GROUP * NK], BF16, tag=f"S{i}", name=f"S{i}")
                     for i in range(2)]
                for h_t in range(2):
                    for ql in range(GROUP):
                        q = g * GROUP + ql
                        nc.vector.tensor_scalar(
                            S[h_t][:, ql * NK:(ql + 1) * NK],
                            ktT_sb[h_t][:],
                            qtT_sb[h_t][:, q:q + 1],
                            None,
                            op0=ALU.add,
                        )
                if g == 0:
                    # fill the pipeline sooner: first tanh needs only 4 adds
                    for cs, ce in ((0, 4), (4, GROUP)):
                        nc.scalar.activation(
                            S[0][:, cs * NK:ce * NK],
                            S[0][:, cs * NK:ce * NK], AF.Tanh,
                        )
                    nc.scalar.activation(S[1][:], S[1][:], AF.Tanh)
                elif g == NGROUPS - 1:
                    # drain the pipeline sooner: matmuls trail each half
                    nc.scalar.activation(S[0][:], S[0][:], AF.Tanh)
                    for cs, ce in ((0, 8), (8, GROUP)):
                        nc.scalar.activation(
                            S[1][:, cs * NK:ce * NK],
                            S[1][:, cs * NK:ce * NK], AF.Tanh,
                        )
                else:
                    for h_t in range(2):
                        nc.scalar.activation(S[h_t][:], S[h_t][:], AF.Tanh)
                for h_t in range(2):
                    for ql in range(GROUP):
                        q = g * GROUP + ql
                        j = (q // 32) * 32
                        jj = q % 32
                        nc.tensor.matmul(
                            prod[j:j + 32, :],
                            ed_bf[:, h_t * 1024 + jj * 32: h_t * 1024 + jj * 32 + 32],
                            S[h_t][:, ql * NK:(ql + 1) * NK],
                            start=(jj == 0 and h_t == 0),
                            stop=(jj == 31 and h_t == 1),
                            tile_position=(0, j),
                        )
                if g == 4:
                    # bank A finished at end of group 3; its exp overlaps
                    # bank B's matmuls (emitted here so it slots between
                    # group 4 and group 5 tanh work on the ACT queue)
                    nc.scalar.activation(
                        expt[0:64, :], prodA[0:64, :], AF.Exp,
                        accum_out=sumexp[0:64, :],
                    )

            # ---- log_softmax tail (bank B + shared ln/identity) -------------
            nc.scalar.activation(
                expt[64:128, :], prodB[64:128, :], AF.Exp,
                accum_out=sumexp[64:128, :],
            )
            nc.scalar.activation(lse[:], sumexp[:], AF.Ln)
            nc.vector.tensor_scalar_mul(neg_lse[:], lse[:], -1.0)
            nc.scalar.activation(
                out_sb[0:64, :], prodA[0:64, :], AF.Identity,
                bias=neg_lse[0:64, 0:1],
            )
            nc.scalar.activation(
                out_sb[64:128, :], prodB[64:128, :], AF.Identity,
                bias=neg_lse[64:128, 0:1],
            )
            nc.sync.dma_start(out_d[:], out_sb[:])

    if split:
        split_multi_waits(nc)
    return nc


def split_multi_waits(nc):
    """walrus codegen accepts at most one sync wait per instruction; move
    extra waits onto same-engine NoOps inserted immediately before."""
    n = 0
    for fn in nc.m.functions:
        for blk in fn.blocks:
            new_insts = []
            for inst in blk.instructions:
                si = inst.sync_info
                if si is not None and len(si.on_wait) > 1:
                    waits = list(si.on_wait)
                    for w in waits[:-1]:
                        nop = mybir.InstNoOp(name=f"WSPLIT-{n}", ins=[], outs=[])
                        n += 1
                        nop.engine = inst.engine
                        nop.sync_info = mybir.SyncInfo(on_wait=[w], on_update=[])
                        new_insts.append(nop)
                    inst.sync_info = mybir.SyncInfo(
                        on_wait=[waits[-1]], on_update=list(si.on_update)
                    )
                new_insts.append(inst)
            if n:
                blk.instructions = new_insts
    return n


def audit_waits(nc, max_waits=1):
    bad = []
    for fn in nc.m.functions:
        for blk in fn.blocks:
            for inst in blk.instructions:
                si = inst.sync_info
                if si is not None and len(si.on_wait) > max_waits:
                    bad.append((inst.name, type(inst).__name__,
                                [w.ant_name for w in si.on_wait]))
    return bad


def make_in_maps(x_query, x_key, w1, w2, v):
    x_query = np.asarray(x_query, dtype=np.float32)
    x_key = np.asarray(x_key, dtype=np.float32)
    w1 = np.asarray(w1, dtype=np.float32)
    w2 = np.asarray(w2, dtype=np.float32)
    v = np.asarray(v, dtype=np.float32).reshape(H)

    w1T = np.ascontiguousarray(w1.T)  # (h_in, o)
    w2T = np.ascontiguousarray(w2.T)

    # ediag[p, h_t*1024 + j*32 + c] = v[h_t*128 + p] if c == j else 0
    ed = np.zeros((128, 2, 32, 32), dtype=np.float32)
    for h_t in range(2):
        for j in range(32):
            ed[:, h_t, j, j] = v[h_t * 128:(h_t + 1) * 128]
    ed = np.ascontiguousarray(ed.reshape(128, ED_F).astype(ml_dtypes.bfloat16))

    in_maps = []
    for c in range(NCORES):
        b = c // 2
        q0 = (c % 2) * QPC
        xqT = np.ascontiguousarray(x_query[b, q0:q0 + QPC, :].T)  # (H, 128)
        xkT = np.ascontiguousarray(x_key[b].T)                    # (H, 512)
        packed_k = np.concatenate(
            [xkT[:128], xkT[128:], w1T[:128], w1T[128:]], axis=1)
        packed_q = np.concatenate(
            [xqT[:128], xqT[128:], w2T[:128], w2T[128:]], axis=1)
        assert packed_k.shape == (128, PKK_F)
        assert packed_q.shape == (128, PKQ_F)
        in_maps.append({
            "packed_k": np.ascontiguousarray(packed_k.astype(ml_dtypes.bfloat16)),
            "packed_q": np.ascontiguousarray(packed_q.astype(ml_dtypes.bfloat16)),
            "ediag": ed,
        })
    return in_maps


_prog_cache = {}


def kernel(x_query, x_key, w1, w2, v):
    if "nc" not in _prog_cache:
        _prog_cache["nc"] = build_program()
    nc = _prog_cache["nc"]
    in_maps = make_in_maps(x_query, x_key, w1, w2, v)
    # A previously-profiled session can leave the device wedged; the failed
    # attempt resets it, so retry a couple of times.
    last_err = None
    for _ in range(3):
        try:
            res = run_bass_kernel_spmd(nc, in_maps, list(range(NCORES)))
            break
        except Exception as e:  # noqa: BLE001 - NRT_EXEC_UNIT_UNRECOVERABLE etc
            last_err = e
    else:
        raise last_err
    out = np.empty((B, NQ, NK), dtype=np.float32)
    for c in range(NCORES):
        b = c // 2
        q0 = (c % 2) * QPC
        out[b, q0:q0 + QPC, :] = res.results[c]["out"]
    return out


if __name__ == "__main__":
    nc = build_program()
    bad = audit_waits(nc)
    if bad:
        print(f"{len(bad)} instructions exceed the 1-wait budget:")
        for name, ty, waits in bad[:20]:
            print(" ", name, ty, waits)
    else:
        print("wait audit OK: all instructions <= 1 sync wait")


# revision 18
# speedup vs baseline: 1.1917x; 1.0010x over previous
"""Trainium2 Bass kernel for nn_AttentionHead (additive/Bahdanau attention).

reference:
    kt = einsum('bkh,oh->bko', x_key, w1)          # (B, NK, H)
    qt = einsum('bqh,oh->bqo', x_query, w2)        # (B, NQ, H)
    prod[b,q,k] = sum_h v[h] * tanh(kt[b,k,h] + qt[b,q,h])
    out = log_softmax(prod, axis=-1)               # (B, NQ, NK)

Shapes: B=4, NQ=256, NK=512, H=256.  8 NeuronCores, data-parallel over
(B x NQ/2): core c handles b = c//2 and a 128-row slice of NQ.

Per-core dataflow:
  - host marshals packed fp32 inputs: transposed xk, xq, w1, w2 plus the
    "ediag" stationaries (for each (h_tile, j in 0..31) a (128,32) matrix,
    zero except column j = v[h_tile*128 : +128]).
  - PE: ktT[o_t] (128, 512) = w1T.T @ xkT       (o on partitions, k free)
        qtT[o_t] (128, 128) = w2T.T @ xqT       (o on partitions, q free)
        ktT cast to bf16 on the PSUM->SBUF copy.
  - DVE: S[h_t][:, q*512:+512] = ktT[h_t] + qtT[h_t][:, q]  (bf16 in/out,
    fp32 per-partition scalar -> high DVE perf mode)
  - ACT: tanh in place on S in large (128, 8192) instructions (the kernel
    bottleneck: 134M tanh elements live on ScalarE only)
  - PE:  prod[q, :] += ediag[h_t, q%32].T @ S[h_t][:, q]  via col-tiled
    matmuls (tile_position=(0, 32j)) accumulating into one PSUM bank
    (q on partitions, k free)
  - log_softmax along free axis, regionized over 32-q row blocks so most
    of it overlaps the main loop: out = prod - ln(sum_k exp(prod));
    |prod| <= sum|v| ~ 8 so skipping max-subtraction is safe in fp32.

walrus only supports ONE sync wait per instruction: split_multi_waits()
post-processes the scheduled IR, moving extra waits onto same-engine
NoOps inserted immediately before the offending instruction.
"""

import sys

sys.path.insert(0, "/opt/trn_rl_repo")

import numpy as np
import ml_dtypes

import concourse.bass as bass
import concourse.mybir as mybir
from concourse import tile
from concourse.bass_utils import run_bass_kernel_spmd

F32 = mybir.dt.float32
BF16 = mybir.dt.bfloat16
AF = mybir.ActivationFunctionType
ALU = mybir.AluOpType

B, NQ, NK, H = 4, 256, 512, 256
NCORES = 8
QPC = (B * NQ) // NCORES  # 128 q rows per core
GROUP = 16                # q's per pipeline group
NGROUPS = QPC // GROUP    # 8

PKK_F = 1536              # xkT (2x512) | w1T (2x256)
PKQ_F = 768               # xqT (2x128) | w2T (2x256)
ED_F = 2 * 32 * 32        # (128, 2048) bf16 v-diag stationaries


def build_program(split=True):
    nc = bass.Bass()

    pkk_d = nc.dram_tensor("packed_k", (128, PKK_F), BF16, kind="ExternalInput")
    pkq_d = nc.dram_tensor("packed_q", (128, PKQ_F), BF16, kind="ExternalInput")
    ed_d = nc.dram_tensor("ediag", (128, ED_F), BF16, kind="ExternalInput")
    out_d = nc.dram_tensor("out", (QPC, NK), F32, kind="ExternalOutput")

    from concourse.tile_rust import add_dep_helper

    with tile.TileContext(nc) as tc:
        with (
            tc.tile_pool(name="const", bufs=1) as cpool,
            tc.tile_pool(name="sadd", bufs=2) as spool,
            tc.tile_pool(name="ppre", bufs=2, space="PSUM") as ppool,
            tc.tile_pool(name="prod", bufs=1, space="PSUM") as prodpool,
        ):
            packed_k = cpool.tile([128, PKK_F], BF16, tag="packed_k")
            packed_q = cpool.tile([128, PKQ_F], BF16, tag="packed_q")
            ed_bf = cpool.tile([128, ED_F], BF16, tag="ed_bf")
            # chain the input DMAs so each gets full HBM bandwidth in
            # criticality order (kt inputs -> qt inputs -> ediag)
            d1 = nc.sync.dma_start(packed_q[:], pkq_d[:])
            d2 = nc.sync.dma_start(packed_k[:], pkk_d[:])
            d3 = nc.sync.dma_start(ed_bf[:], ed_d[:])
            add_dep_helper(d2.ins, d1.ins, True, "serialize input DMAs")
            add_dep_helper(d3.ins, d2.ins, True, "serialize input DMAs")

            def xkT(i):
                return packed_k[:, i * NK:(i + 1) * NK]

            def w1T(i, o):
                return packed_k[:, 1024 + i * 256 + o * 128:1024 + i * 256 + (o + 1) * 128]

            def xqT(i):
                return packed_q[:, i * QPC:(i + 1) * QPC]

            def w2T(i, o):
                return packed_q[:, 256 + i * 256 + o * 128:256 + i * 256 + (o + 1) * 128]

            # ---- ktT / qtT (PSUM->SBUF copies on the idle ACT engine) -------
            ktT_sb = [cpool.tile([128, NK], BF16, tag=f"ktT{o}", name=f"ktT{o}")
                      for o in range(2)]
            qtT_sb = [cpool.tile([128, QPC], F32, tag=f"qtT{o}", name=f"qtT{o}")
                      for o in range(2)]
            for o_t in range(2):
                pq = ppool.tile([128, QPC], F32, tag="pq", name="pq")
                for h_t in range(2):
                    nc.tensor.matmul(
                        pq[:], w2T(h_t, o_t), xqT(h_t),
                        start=(h_t == 0), stop=(h_t == 1),
                    )
                nc.vector.tensor_copy(qtT_sb[o_t][:], pq[:])
            for o_t in range(2):
                pk = ppool.tile([128, NK], F32, tag="pk", name="pk")
                for h_t in range(2):
                    nc.tensor.matmul(
                        pk[:], w1T(h_t, o_t), xkT(h_t),
                        start=(h_t == 0), stop=(h_t == 1),
                    )
                nc.vector.tensor_copy(ktT_sb[o_t][:], pk[:])

            # ---- main loop ---------------------------------------------------
            # Two PSUM banks for prod: q 0..63 in bank A partitions 0..63,
            # q 64..127 in bank B partitions 64..127 -> the bank-A softmax
            # overlaps the bank-B matmuls (PSUM deps are bank-granular).
            prodA = prodpool.tile([128, NK], F32, tag="prodA", name="prodA")
            prodB = prodpool.tile([128, NK], F32, tag="prodB", name="prodB")
            sumexp = cpool.tile([128, 1], F32, tag="sumexp")
            lse = cpool.tile([128, 1], F32, tag="lse")
            neg_lse = cpool.tile([128, 1], F32, tag="neg_lse")
            expt = cpool.tile([128, NK], F32, tag="expt")
            out_sb = cpool.tile([128, NK], F32, tag="out_sb")

            for g in range(NGROUPS):
                prod = prodA if g < 4 else prodB
                S = [spool.tile([128, GROUP * NK], BF16, tag=f"S{i}", name=f"S{i}")
                     for i in range(2)]
                for h_t in range(2):
                    for ql in range(GROUP):
                        q = g * GROUP + ql
                        nc.vector.tensor_scalar(
                            S[h_t][:, ql * NK:(ql + 1) * NK],
                            ktT_sb[h_t][:],
                            qtT_sb[h_t][:, q:q + 1],
                            None,
                            op0=ALU.add,
                        )
                if g == 0:
                    # fill the pipeline sooner: first tanh needs only 4 adds
                    for cs, ce in ((0, 4), (4, 10), (10, GROUP)):
                        nc.scalar.activation(
                            S[0][:, cs * NK:ce * NK],
                            S[0][:, cs * NK:ce * NK], AF.Tanh,
                        )
                    nc.scalar.activation(S[1][:], S[1][:], AF.Tanh)
                elif g == NGROUPS - 1:
                    # drain the pipeline sooner: matmuls trail each half
                    nc.scalar.activation(S[0][:], S[0][:], AF.Tanh)
                    for cs, ce in ((0, 8), (8, 12), (12, GROUP)):
                        nc.scalar.activation(
                            S[1][:, cs * NK:ce * NK],
                            S[1][:, cs * NK:ce * NK], AF.Tanh,
                        )
                else:
                    for h_t in range(2):
                        nc.scalar.activation(S[h_t][:], S[h_t][:], AF.Tanh)
                for h_t in range(2):
                    for ql in range(GROUP):
                        q = g * GROUP + ql
                        j = (q // 32) * 32
                        jj = q % 32
                        nc.tensor.matmul(
                            prod[j:j + 32, :],
                            ed_bf[:, h_t * 1024 + jj * 32: h_t * 1024 + jj * 32 + 32],
                            S[h_t][:, ql * NK:(ql + 1) * NK],
                            start=(jj == 0 and h_t == 0),
                            stop=(jj == 31 and h_t == 1),
                            tile_position=(0, j),
                        )
                if g == 4:
                    # bank A finished at end of group 3; its exp overlaps
                    # bank B's matmuls (emitted here so it slots between
                    # group 4 and group 5 tanh work on the ACT queue)
                    nc.scalar.activation(
                        expt[0:64, :], prodA[0:64, :], AF.Exp,
                        accum_out=sumexp[0:64, :],
                    )

            # ---- log_softmax tail (bank B + shared ln/identity) -------------
            nc.scalar.activation(
                expt[64:128, :], prodB[64:128, :], AF.Exp,
                accum_out=sumexp[64:128, :],
            )
            nc.scalar.activation(lse[:], sumexp[:], AF.Ln)
            nc.vector.tensor_scalar_mul(neg_lse[:], lse[:], -1.0)
            nc.scalar.activation(
                out_sb[0:64, :], prodA[0:64, :], AF.Identity,
                bias=neg_lse[0:64, 0:1],
            )
            nc.scalar.activation(
                out_sb[64:128, :], prodB[64:128, :], AF.Identity,
                bias=neg_lse[64:128, 0:1],
            )
            nc.sync.dma_start(out_d[:], out_sb[:])

    if split:
        split_multi_waits(nc)
    return nc


def split_multi_waits(nc):
    """walrus codegen accepts at most one sync wait per instruction; move
    extra waits onto same-engine NoOps inserted immediately before."""
    n = 0
    for fn in nc.m.functions:
        for blk in fn.blocks:
            new_insts = []
            for inst in blk.instructions:
                si = inst.sync_info
                if si is not None and len(si.on_wait) > 1:
                    waits = list(si.on_wait)
                    for w in waits[:-1]:
                        nop = mybir.InstNoOp(name=f"WSPLIT-{n}", ins=[], outs=[])
                        n += 1
                        nop.engine = inst.engine
                        nop.sync_info = mybir.SyncInfo(on_wait=[w], on_update=[])
                        new_insts.append(nop)
                    inst.sync_info = mybir.SyncInfo(
                        on_wait=[waits[-1]], on_update=list(si.on_update)
                    )
                new_insts.append(inst)
            if n:
                blk.instructions = new_insts
    return n


def audit_waits(nc, max_waits=1):
    bad = []
    for fn in nc.m.functions:
        for blk in fn.blocks:
            for inst in blk.instructions:
                si = inst.sync_info
                if si is not None and len(si.on_wait) > max_waits:
                    bad.append((inst.name, type(inst).__name__,
                                [w.ant_name for w in si.on_wait]))
    return bad


def make_in_maps(x_query, x_key, w1, w2, v):
    x_query = np.asarray(x_query, dtype=np.float32)
    x_key = np.asarray(x_key, dtype=np.float32)
    w1 = np.asarray(w1, dtype=np.float32)
    w2 = np.asarray(w2, dtype=np.float32)
    v = np.asarray(v, dtype=np.float32).reshape(H)

    w1T = np.ascontiguousarray(w1.T)  # (h_in, o)
    w2T = np.ascontiguousarray(w2.T)

    # ediag[p, h_t*1024 + j*32 + c] = v[h_t*128 + p] if c == j else 0
    ed = np.zeros((128, 2, 32, 32), dtype=np.float32)
    for h_t in range(2):
        for j in range(32):
            ed[:, h_t, j, j] = v[h_t * 128:(h_t + 1) * 128]
    ed = np.ascontiguousarray(ed.reshape(128, ED_F).astype(ml_dtypes.bfloat16))

    in_maps = []
    for c in range(NCORES):
        b = c // 2
        q0 = (c % 2) * QPC
        xqT = np.ascontiguousarray(x_query[b, q0:q0 + QPC, :].T)  # (H, 128)
        xkT = np.ascontiguousarray(x_key[b].T)                    # (H, 512)
        packed_k = np.concatenate(
            [xkT[:128], xkT[128:], w1T[:128], w1T[128:]], axis=1)
        packed_q = np.concatenate(
            [xqT[:128], xqT[128:], w2T[:128], w2T[128:]], axis=1)
        assert packed_k.shape == (128, PKK_F)
        assert packed_q.shape == (128, PKQ_F)
        in_maps.append({
            "packed_k": np.ascontiguousarray(packed_k.astype(ml_dtypes.bfloat16)),
            "packed_q": np.ascontiguousarray(packed_q.astype(ml_dtypes.bfloat16)),
            "ediag": ed,
        })
    return in_maps


_prog_cache = {}


def kernel(x_query, x_key, w1, w2, v):
    if "nc" not in _prog_cache:
        _prog_cache["nc"] = build_program()
    nc = _prog_cache["nc"]
    in_maps = make_in_maps(x_query, x_key, w1, w2, v)
    # A previously-profiled session can leave the device wedged; the failed
    # attempt resets it, so retry a couple of times.
    last_err = None
    for _ in range(3):
        try:
            res = run_bass_kernel_spmd(nc, in_maps, list(range(NCORES)))
            break
        except Exception as e:  # noqa: BLE001 - NRT_EXEC_UNIT_UNRECOVERABLE etc
            last_err = e
    else:
        raise last_err
    out = np.empty((B, NQ, NK), dtype=np.float32)
    for c in range(NCORES):
        b = c // 2
        q0 = (c % 2) * QPC
        out[b, q0:q0 + QPC, :] = res.results[c]["out"]
    return out


if __name__ == "__main__":
    nc = build_program()
    bad = audit_waits(nc)
    if bad:
        print(f"{len(bad)} instructions exceed the 1-wait budget:")
        for name, ty, waits in bad[:20]:
            print(" ", name, ty, waits)
    else:
        print("wait audit OK: all instructions <= 1 sync wait")


# revision 19
# speedup vs baseline: 1.2196x; 1.0234x over previous
"""Trainium2 Bass kernel for nn_AttentionHead (additive/Bahdanau attention).

reference:
    kt = einsum('bkh,oh->bko', x_key, w1)          # (B, NK, H)
    qt = einsum('bqh,oh->bqo', x_query, w2)        # (B, NQ, H)
    prod[b,q,k] = sum_h v[h] * tanh(kt[b,k,h] + qt[b,q,h])
    out = log_softmax(prod, axis=-1)               # (B, NQ, NK)

Shapes: B=4, NQ=256, NK=512, H=256.  8 NeuronCores, data-parallel over
(B x NQ/2): core c handles b = c//2 and a 128-row slice of NQ.

Per-core dataflow:
  - host marshals packed fp32 inputs: transposed xk, xq, w1, w2 plus the
    "ediag" stationaries (for each (h_tile, j in 0..31) a (128,32) matrix,
    zero except column j = v[h_tile*128 : +128]).
  - PE: ktT[o_t] (128, 512) = w1T.T @ xkT       (o on partitions, k free)
        qtT[o_t] (128, 128) = w2T.T @ xqT       (o on partitions, q free)
        ktT cast to bf16 on the PSUM->SBUF copy.
  - DVE: S[h_t][:, q*512:+512] = ktT[h_t] + qtT[h_t][:, q]  (bf16 in/out,
    fp32 per-partition scalar -> high DVE perf mode)
  - ACT: tanh in place on S in large (128, 8192) instructions (the kernel
    bottleneck: 134M tanh elements live on ScalarE only)
  - PE:  prod[q, :] += ediag[h_t, q%32].T @ S[h_t][:, q]  via col-tiled
    matmuls (tile_position=(0, 32j)) accumulating into one PSUM bank
    (q on partitions, k free)
  - log_softmax along free axis, regionized over 32-q row blocks so most
    of it overlaps the main loop: out = prod - ln(sum_k exp(prod));
    |prod| <= sum|v| ~ 8 so skipping max-subtraction is safe in fp32.

walrus only supports ONE sync wait per instruction: split_multi_waits()
post-processes the scheduled IR, moving extra waits onto same-engine
NoOps inserted immediately before the offending instruction.
"""

import sys

sys.path.insert(0, "/opt/trn_rl_repo")

import numpy as np
import ml_dtypes

import concourse.bass as bass
import concourse.mybir as mybir
from concourse import tile
from concourse.bass_utils import run_bass_kernel_spmd

F32 = mybir.dt.float32
BF16 = mybir.dt.bfloat16
AF = mybir.ActivationFunctionType
ALU = mybir.AluOpType

B, NQ, NK, H = 4, 256, 512, 256
NCORES = 8
QPC = (B * NQ) // NCORES  # 128 q rows per core
GROUP = 16                # q's per pipeline group
NGROUPS = QPC // GROUP    # 8

PKK_F = 1536              # xkT (2x512) | w1T (2x256)
PKQ_F = 768               # xqT (2x128) | w2T (2x256)
ED_F = 2 * 32 * 32        # (128, 2048) bf16 v-diag stationaries


def build_program(split=True):
    nc = bass.Bass()

    pkk_d = nc.dram_tensor("packed_k", (128, PKK_F), BF16, kind="ExternalInput")
    pkq_d = nc.dram_tensor("packed_q", (128, PKQ_F), BF16, kind="ExternalInput")
    ed_d = nc.dram_tensor("ediag", (128, ED_F), BF16, kind="ExternalInput")
    out_d = nc.dram_tensor("out", (QPC, NK), F32, kind="ExternalOutput")

    from concourse.tile_rust import add_dep_helper

    with tile.TileContext(nc) as tc:
        with (
            tc.tile_pool(name="const", bufs=1) as cpool,
            tc.tile_pool(name="sadd", bufs=2) as spool,
            tc.tile_pool(name="ppre", bufs=2, space="PSUM") as ppool,
            tc.tile_pool(name="prod", bufs=1, space="PSUM") as prodpool,
        ):
            packed_k = cpool.tile([128, PKK_F], BF16, tag="packed_k")
            packed_q = cpool.tile([128, PKQ_F], BF16, tag="packed_q")
            ed_bf = cpool.tile([128, ED_F], BF16, tag="ed_bf")
            # chain the input DMAs so each gets full HBM bandwidth in
            # criticality order (kt inputs -> qt inputs -> ediag)
            d1 = nc.sync.dma_start(packed_k[:], pkk_d[:])
            d2 = nc.sync.dma_start(packed_q[:], pkq_d[:])
            d3 = nc.sync.dma_start(ed_bf[:], ed_d[:])
            add_dep_helper(d2.ins, d1.ins, True, "serialize input DMAs")
            add_dep_helper(d3.ins, d2.ins, True, "serialize input DMAs")

            def xkT(i):
                return packed_k[:, i * NK:(i + 1) * NK]

            def w1T(i, o):
                return packed_k[:, 1024 + i * 256 + o * 128:1024 + i * 256 + (o + 1) * 128]

            def xqT(i):
                return packed_q[:, i * QPC:(i + 1) * QPC]

            def w2T(i, o):
                return packed_q[:, 256 + i * 256 + o * 128:256 + i * 256 + (o + 1) * 128]

            # ---- ktT / qtT (PSUM->SBUF copies on the idle ACT engine) -------
            ktT_sb = [cpool.tile([128, NK], BF16, tag=f"ktT{o}", name=f"ktT{o}")
                      for o in range(2)]
            qtT_sb = [cpool.tile([128, QPC], F32, tag=f"qtT{o}", name=f"qtT{o}")
                      for o in range(2)]
            for o_t in range(2):
                pq = ppool.tile([128, QPC], F32, tag="pq", name="pq")
                for h_t in range(2):
                    nc.tensor.matmul(
                        pq[:], w2T(h_t, o_t), xqT(h_t),
                        start=(h_t == 0), stop=(h_t == 1),
                    )
                nc.vector.tensor_copy(qtT_sb[o_t][:], pq[:])
            for o_t in range(2):
                pk = ppool.tile([128, NK], F32, tag="pk", name="pk")
                for h_t in range(2):
                    nc.tensor.matmul(
                        pk[:], w1T(h_t, o_t), xkT(h_t),
                        start=(h_t == 0), stop=(h_t == 1),
                    )
                nc.vector.tensor_copy(ktT_sb[o_t][:], pk[:])

            # ---- main loop ---------------------------------------------------
            # Two PSUM banks for prod: q 0..63 in bank A partitions 0..63,
            # q 64..127 in bank B partitions 64..127 -> the bank-A softmax
            # overlaps the bank-B matmuls (PSUM deps are bank-granular).
            prodA = prodpool.tile([128, NK], F32, tag="prodA", name="prodA")
            prodB = prodpool.tile([128, NK], F32, tag="prodB", name="prodB")
            sumexp = cpool.tile([128, 1], F32, tag="sumexp")
            lse = cpool.tile([128, 1], F32, tag="lse")
            neg_lse = cpool.tile([128, 1], F32, tag="neg_lse")
            expt = cpool.tile([128, NK], F32, tag="expt")
            out_sb = cpool.tile([128, NK], F32, tag="out_sb")

            for g in range(NGROUPS):
                prod = prodA if g < 4 else prodB
                S = [spool.tile([128, GROUP * NK], BF16, tag=f"S{i}", name=f"S{i}")
                     for i in range(2)]
                for h_t in range(2):
                    for ql in range(GROUP):
                        q = g * GROUP + ql
                        nc.vector.tensor_scalar(
                            S[h_t][:, ql * NK:(ql + 1) * NK],
                            ktT_sb[h_t][:],
                            qtT_sb[h_t][:, q:q + 1],
                            None,
                            op0=ALU.add,
                        )
                if g == 0:
                    # fill the pipeline sooner: first tanh needs only 4 adds
                    for cs, ce in ((0, 4), (4, 10), (10, GROUP)):
                        nc.scalar.activation(
                            S[0][:, cs * NK:ce * NK],
                            S[0][:, cs * NK:ce * NK], AF.Tanh,
                        )
                    nc.scalar.activation(S[1][:], S[1][:], AF.Tanh)
                elif g == NGROUPS - 1:
                    # drain the pipeline sooner: matmuls trail each half
                    nc.scalar.activation(S[0][:], S[0][:], AF.Tanh)
                    for cs, ce in ((0, 8), (8, 12), (12, GROUP)):
                        nc.scalar.activation(
                            S[1][:, cs * NK:ce * NK],
                            S[1][:, cs * NK:ce * NK], AF.Tanh,
                        )
                else:
                    for h_t in range(2):
                        nc.scalar.activation(S[h_t][:], S[h_t][:], AF.Tanh)
                for h_t in range(2):
                    for ql in range(GROUP):
                        q = g * GROUP + ql
                        j = (q // 32) * 32
                        jj = q % 32
                        nc.tensor.matmul(
                            prod[j:j + 32, :],
                            ed_bf[:, h_t * 1024 + jj * 32: h_t * 1024 + jj * 32 + 32],
                            S[h_t][:, ql * NK:(ql + 1) * NK],
                            start=(jj == 0 and h_t == 0),
                            stop=(jj == 31 and h_t == 1),
                            tile_position=(0, j),
                        )
                if g == 4:
                    # bank A finished at end of group 3; its exp overlaps
                    # bank B's matmuls (emitted here so it slots between
                    # group 4 and group 5 tanh work on the ACT queue)
                    nc.scalar.activation(
                        expt[0:64, :], prodA[0:64, :], AF.Exp,
                        accum_out=sumexp[0:64, :],
                    )

            # ---- log_softmax tail (bank B + shared ln/identity) -------------
            nc.scalar.activation(
                expt[64:128, :], prodB[64:128, :], AF.Exp,
                accum_out=sumexp[64:128, :],
            )
            nc.scalar.activation(lse[:], sumexp[:], AF.Ln)
            nc.vector.tensor_scalar_mul(neg_lse[:], lse[:], -1.0)
            nc.scalar.activation(
                out_sb[0:64, :], prodA[0:64, :], AF.Identity,
                bias=neg_lse[0:64, 0:1],
            )
            nc.scalar.activation(
                out_sb[64:128, :], prodB[64:128, :], AF.Identity,
                bias=neg_lse[64:128, 0:1],
            )
            nc.sync.dma_start(out_d[:], out_sb[:])

    if split:
        split_multi_waits(nc)
    return nc


def split_multi_waits(nc):
    """walrus codegen accepts at most one sync wait per instruction; move
    extra waits onto same-engine NoOps inserted immediately before."""
    n = 0
    for fn in nc.m.functions:
        for blk in fn.blocks:
            new_insts = []
            for inst in blk.instructions:
                si = inst.sync_info
                if si is not None and len(si.on_wait) > 1:
                    waits = list(si.on_wait)
                    for w in waits[:-1]:
                        nop = mybir.InstNoOp(name=f"WSPLIT-{n}", ins=[], outs=[])
                        n += 1
                        nop.engine = inst.engine
                        nop.sync_info = mybir.SyncInfo(on_wait=[w], on_update=[])
                        new_insts.append(nop)
                    inst.sync_info = mybir.SyncInfo(
                        on_wait=[waits[-1]], on_update=list(si.on_update)
                    )
                new_insts.append(inst)
            if n:
                blk.instructions = new_insts
    return n


def audit_waits(nc, max_waits=1):
    bad = []
    for fn in nc.m.functions:
        for blk in fn.blocks:
            for inst in blk.instructions:
                si = inst.sync_info
                if si is not None and len(si.on_wait) > max_waits:
                    bad.append((inst.name, type(inst).__name__,
                                [w.ant_name for w in si.on_wait]))
    return bad


def make_in_maps(x_query, x_key, w1, w2, v):
    x_query = np.asarray(x_query, dtype=np.float32)
    x_key = np.asarray(x_key, dtype=np.float32)
    w1 = np.asarray(w1, dtype=np.float32)
    w2 = np.asarray(w2, dtype=np.float32)
    v = np.asarray(v, dtype=np.float32).reshape(H)

    w1T = np.ascontiguousarray(w1.T)  # (h_in, o)
    w2T = np.ascontiguousarray(w2.T)

    # ediag[p, h_t*1024 + j*32 + c] = v[h_t*128 + p] if c == j else 0
    ed = np.zeros((128, 2, 32, 32), dtype=np.float32)
    for h_t in range(2):
        for j in range(32):
            ed[:, h_t, j, j] = v[h_t * 128:(h_t + 1) * 128]
    ed = np.ascontiguousarray(ed.reshape(128, ED_F).astype(ml_dtypes.bfloat16))

    in_maps = []
    for c in range(NCORES):
        b = c // 2
        q0 = (c % 2) * QPC
        xqT = np.ascontiguousarray(x_query[b, q0:q0 + QPC, :].T)  # (H, 128)
        xkT = np.ascontiguousarray(x_key[b].T)                    # (H, 512)
        packed_k = np.concatenate(
            [xkT[:128], xkT[128:], w1T[:128], w1T[128:]], axis=1)
        packed_q = np.concatenate(
            [xqT[:128], xqT[128:], w2T[:128], w2T[128:]], axis=1)
        assert packed_k.shape == (128, PKK_F)
        assert packed_q.shape == (128, PKQ_F)
        in_maps.append({
            "packed_k": np.ascontiguousarray(packed_k.astype(ml_dtypes.bfloat16)),
            "packed_q": np.ascontiguousarray(packed_q.astype(ml_dtypes.bfloat16)),
            "ediag": ed,
        })
    return in_maps


_prog_cache = {}


def kernel(x_query, x_key, w1, w2, v):
    if "nc" not in _prog_cache:
        _prog_cache["nc"] = build_program()
    nc = _prog_cache["nc"]
    in_maps = make_in_maps(x_query, x_key, w1, w2, v)
    # A previously-profiled session can leave the device wedged; the failed
    # attempt resets it, so retry a couple of times.
    last_err = None
    for _ in range(3):
        try:
            res = run_bass_kernel_spmd(nc, in_maps, list(range(NCORES)))
            break
        except Exception as e:  # noqa: BLE001 - NRT_EXEC_UNIT_UNRECOVERABLE etc
            last_err = e
    else:
        raise last_err
    out = np.empty((B, NQ, NK), dtype=np.float32)
    for c in range(NCORES):
        b = c // 2
        q0 = (c % 2) * QPC
        out[b, q0:q0 + QPC, :] = res.results[c]["out"]
    return out


if __name__ == "__main__":
    nc = build_program()
    bad = audit_waits(nc)
    if bad:
        print(f"{len(bad)} instructions exceed the 1-wait budget:")
        for name, ty, waits in bad[:20]:
            print(" ", name, ty, waits)
    else:
        print("wait audit OK: all instructions <= 1 sync wait")
